# revision 6
# baseline (speedup 1.0000x reference)
"""AttentionFusion kernel for 8x TRN2 NeuronCores.

Math per batch element b (one core each, data-parallel over B=8):
    q  = x[b]            [C=512, L=4096]
    kv = concat(spatial_feat[b], multi_scale_feat[b])   [2C=1024, L]
    attn  = softmax(s * q @ kv^T)          s = scale / sqrt(L)
    out   = conv_w @ (attn @ kv) + conv_b  [C, L]

Reformulated to cut work + transposes:
    out = (conv_w' @ attnE) @ kv,  where attnE = exp(s*q@kv^T)
    conv_w'[o,c] = conv_w[o,c] / rowsum[c]   (softmax normalization folded
    into the tiny conv weight; no rowmax subtraction -- logits are ~N(0,1)
    after the 1/sqrt(L) scale so exp() cannot overflow)

v2 schedule (vs v1):
  - kv transposes moved off TensorE onto the DMA xbar
    (dma_start(transpose=True), 16x128 tiles) so PE only runs q/w
    transposes + the three matmul chains.
  - mm1 is split into two k-halves (sp then ms) with all four 128-row
    attn PSUM tiles live per half, so PE consumes kvT l-chunks in the
    order the DMA stream delivers them.
  - softmax has no max pass; per-half row sums come from the Exp
    activation's accum_out and are combined at the end.
  - output is stored bf16 (cast to f32 on host) to halve store DMA.

Engine usage:
  - f32->bf16 input casts inside SWDGE DMA (gpsimd queue).
  - kv SBUF->SBUF bf16 transposes on the HWDGE xbar (sync queue).
  - q/conv_w transposes on TensorE via PSUM, drained by ACT/DVE.
  - matmuls (bf16 in, f32 PSUM):
      mm1: attn[c,k] += qT[l,c].T @ kvT[l,k]         (accum over l)
      wa : waT[k,o]  += attnE[c,k].T @ conv_w'T[c,o] (accum over c)
      mm2: out[o,l]  += waT[k,o].T @ kv[k,l]         (accum over k)
  - exp with accum row-sum on ACT; recip + conv_w scale on DVE.
"""

import numpy as np

B, C, H, W = 8, 512, 64, 64
L = H * W            # 4096
G = (2 * C) // 128   # 8 kv partition groups
M = C // 128         # 4 row blocks
LJ = L // 128        # 32 l-chunks
NCORES = 8

_cache = {}


def _build():
    import concourse.bass as bass
    import concourse.mybir as mybir
    import concourse.tile as tile
    from concourse import bacc
    from concourse.masks import make_identity

    F32 = mybir.dt.float32
    BF16 = mybir.dt.bfloat16
    AF = mybir.ActivationFunctionType

    nc = bacc.Bacc("TRN2", target_bir_lowering=False, debug=False,
                   num_devices=NCORES)
    q_d = nc.dram_tensor("q", [C, L], F32, kind="ExternalInput")
    sp_d = nc.dram_tensor("sp", [C, L], F32, kind="ExternalInput")
    ms_d = nc.dram_tensor("ms", [C, L], F32, kind="ExternalInput")
    w_d = nc.dram_tensor("conv_w", [C, C], F32, kind="ExternalInput")
    b_d = nc.dram_tensor("conv_b", [C], F32, kind="ExternalInput")
    s_d = nc.dram_tensor("scale", [1], F32, kind="ExternalInput")
    out_d = nc.dram_tensor("out", [C, L], BF16, kind="ExternalOutput")

    def drain(i, dst, src):
        # alternate PSUM->SBUF drains between ACT and DVE
        if i % 2 == 0:
            nc.scalar.copy(dst, src)
        else:
            nc.vector.tensor_copy(out=dst, in_=src)

    with tile.TileContext(nc) as tc:
        with tc.tile_pool(name="big", bufs=1) as big, \
             tc.tile_pool(name="qn", bufs=2) as qn_pool, \
             tc.tile_pool(name="outsb", bufs=4) as out_pool, \
             tc.tile_pool(name="sm", bufs=12) as sm:

            # ---------- constants ----------
            ident = big.tile([128, 128], BF16)
            make_identity(nc, ident)

            s_ap = s_d.ap()
            s_bcast = bass.AP(tensor=s_ap.tensor, offset=s_ap.offset,
                              ap=[[0, 128]] + list(s_ap.ap))
            s_sb = big.tile([128, 1], F32)
            nc.sync.dma_start(out=s_sb, in_=s_bcast)
            s2 = big.tile([128, 1], F32)            # scale * L^-0.5
            nc.scalar.mul(s2, s_sb, float(L) ** -0.5)

            bias_sb = big.tile([128, M], F32)
            nc.gpsimd.dma_start(out=bias_sb,
                                in_=b_d.ap().rearrange("(mo p) -> p mo",
                                                       p=128))

            w_nat = big.tile([128, M, C], BF16)      # conv_w[128*ob+p, c]
            nc.gpsimd.dma_start(out=w_nat,
                                in_=w_d.ap().rearrange("(ob p) c -> p ob c",
                                                       p=128))

            # ---------- persistent SBUF tensors ----------
            kv = big.tile([128, G, L], BF16)         # kv[128g+p, l]
            kvT = big.tile([128, LJ, G, 128], BF16)  # kv[128g+kk, 128j+p]
            qTs = [big.tile([128, LJ, 128], BF16, name=f"qT{m}")
                   for m in range(M)]                # q[128m+c, 128j+p]
            attnE = big.tile([128, M, 2 * C], BF16)
            wT = big.tile([128, M, C], BF16)         # conv_w[o, 128cb+p]
            wTp = w_nat                              # wT * recip[c]; reuses
            # w_nat's storage -- w_nat is dead once wT is built.
            waT = big.tile([128, G, C], BF16)
            recip = big.tile([128, M], F32)

            # ---------- DMA program ----------
            # gpsimd (SWDGE cast) queue order = delivery order:
            #   w, bias, q0, sp(lq0), q1, sp(lq1), q2, sp(lq2), q3,
            #   sp(lq3), ms(lq0..3)
            # sync (HWDGE) queue: kv xbar transposes chasing the loads.
            q_nats = {}

            def load_q(m):
                t = qn_pool.tile([128, L], BF16, name=f"qnat{m}", tag="qnat")
                nc.gpsimd.dma_start(out=t, in_=q_d.ap()[128 * m:128 * (m + 1), :])
                q_nats[m] = t

            def load_kv_chunk(g, lq):
                src = sp_d if g < M else ms_d
                r0 = 128 * (g % M)
                ls = slice(1024 * lq, 1024 * (lq + 1))
                nc.gpsimd.dma_start(out=kv[:, g, ls],
                                    in_=src.ap()[r0:r0 + 128, ls])

            def xbar_kv_chunk(g, lq):
                ls = slice(1024 * lq, 1024 * (lq + 1))
                nc.sync.dma_start(out=kvT[:, 8 * lq:8 * (lq + 1), g, :],
                                  in_=kv[:, g, ls], transpose=True)

            def emit_kv_lq(half, lq):
                for gl in range(4):
                    g = 4 * half + gl
                    load_kv_chunk(g, lq)
                    xbar_kv_chunk(g, lq)

            # DMA delivery order: w, q0, q1, sp0, sp1, q2, sp2, q3, sp3,
            # ms0..ms3.  q blocks early so PE has transpose work while the
            # kv stream ramps; q2/q3 slotted so their qT units line up.
            load_q(0)
            load_q(1)
            emit_kv_lq(0, 0)
            emit_kv_lq(0, 1)
            load_q(2)
            emit_kv_lq(0, 2)
            load_q(3)
            emit_kv_lq(0, 3)
            for lq in range(4):
                emit_kv_lq(1, lq)

            # ---------- PE program ----------
            with tc.tile_pool(name="tp", bufs=2, space="PSUM") as tp_pool, \
                 tc.tile_pool(name="aps", bufs=5, space="PSUM") as attn_ps:

                # conv_w transpose: wT[p,cb,o] = w[o, 128cb+p]
                for cb in range(M):
                    tp = tp_pool.tile([128, 512], BF16, name=f"tpw{cb}",
                                      tag="tp")
                    for ob in range(M):
                        nc.tensor.transpose(
                            tp[:, 128 * ob:128 * (ob + 1)],
                            w_nat[:, ob, 128 * cb:128 * (cb + 1)], ident)
                    drain(cb, wT[:, cb, :], tp)

                ci = 0

                def qT_all(m):
                    # transpose all 32 l-tiles of q block m right after its
                    # load so the qn_pool buffer frees early (avoids a
                    # DMA-queue/PE-queue deadlock cycle)
                    nonlocal ci
                    for lq in range(4):
                        tp = tp_pool.tile([128, 1024], BF16,
                                          name=f"tpq{m}_{lq}", tag="tp")
                        for i in range(8):
                            j = 8 * lq + i
                            nc.tensor.transpose(
                                tp[:, 128 * i:128 * (i + 1)],
                                q_nats[m][:, 128 * j:128 * (j + 1)], ident)
                        drain(ci, qTs[m][:, 8 * lq:8 * (lq + 1), :], tp)
                        ci += 1

                attn_t = {}

                def mm1_step(half, m, lq):
                    # 8 j-matmuls accumulating attn[m, k-half] over l
                    gs = slice(4 * half, 4 * half + 4)
                    t = attn_t[m]
                    for i in range(8):
                        j = 8 * lq + i
                        nc.tensor.matmul(t, lhsT=qTs[m][:, j, :],
                                         rhs=kvT[:, j, gs, :],
                                         start=(j == 0), stop=(j == LJ - 1))

                def exp_half(half, m):
                    # attnE[:, m, half] = exp(s2 * attn), rowsum -> accum
                    ks = slice(512 * half, 512 * (half + 1))
                    rs = sm.tile([128, 1], F32, name=f"rs{half}_{m}",
                                 tag="sm")
                    nc.scalar.activation(out=attnE[:, m, ks], in_=attn_t[m],
                                         func=AF.Exp, scale=s2, accum_out=rs)
                    return rs

                rsA = {}

                # ---- half A (sp, k 0:512) ----
                # PE order matched to DMA delivery: q0,q1 transposes first,
                # then mm1 chunks as kvT lq-batches land, q2/q3 transposes
                # slotted where their loads complete.
                for m in range(M):
                    attn_t[m] = attn_ps.tile([128, 512], F32,
                                             name=f"attnA{m}", tag="attn")
                qT_all(0)
                qT_all(1)
                mm1_step(0, 0, 0)
                mm1_step(0, 1, 0)
                mm1_step(0, 0, 1)
                mm1_step(0, 1, 1)
                qT_all(2)
                mm1_step(0, 2, 0)
                mm1_step(0, 2, 1)
                mm1_step(0, 0, 2)
                mm1_step(0, 1, 2)
                mm1_step(0, 2, 2)
                qT_all(3)
                mm1_step(0, 3, 0)
                mm1_step(0, 3, 1)
                mm1_step(0, 3, 2)
                for m in range(M):
                    mm1_step(0, m, 3)
                for m in range(M):
                    rsA[m] = exp_half(0, m)

                # ---- half B (ms, k 512:1024) ----
                for m in range(M):
                    attn_t[m] = attn_ps.tile([128, 512], F32,
                                             name=f"attnB{m}", tag="attn")
                for lq in range(4):
                    for m in range(M):
                        mm1_step(1, m, lq)
                for m in range(M):
                    rsB = exp_half(1, m)
                    rs = sm.tile([128, 1], F32, name=f"rsT{m}", tag="sm")
                    nc.vector.tensor_add(out=rs, in0=rsA[m], in1=rsB)
                    nc.vector.reciprocal(out=recip[:, m:m + 1], in_=rs)
                    nc.vector.tensor_scalar_mul(wTp[:, m, :], wT[:, m, :],
                                                recip[:, m:m + 1])

            # ---- wa: waT[k,o] = sum_c attnE_norm[c,k] * wTp[c,o] ----
            with tc.tile_pool(name="wps", bufs=2, space="PSUM") as wa_ps, \
                 tc.tile_pool(name="ops", bufs=4, space="PSUM") as out_ps:
                for g in range(G):
                    wa_t = wa_ps.tile([128, C], F32, name=f"wa{g}", tag="wa")
                    for cb in range(M):
                        nc.tensor.matmul(
                            wa_t, lhsT=attnE[:, cb, 128 * g:128 * (g + 1)],
                            rhs=wTp[:, cb, :],
                            start=(cb == 0), stop=(cb == M - 1))
                    drain(g, waT[:, g, :], wa_t)

                # ---- mm2: out[o,l] = sum_k waT[k,o]*kv[k,l] (+bias) ----
                for mo in range(M):
                    for lh in range(2):             # quads of l-tiles
                        acc = [out_ps.tile([128, 512], F32,
                                           name=f"acc{mo}_{lh}_{i}",
                                           tag="acc")
                               for i in range(4)]
                        for g in range(G):
                            lhsT = waT[:, g, 128 * mo:128 * (mo + 1)]
                            for i in range(4):
                                lt = 4 * lh + i
                                nc.tensor.matmul(
                                    acc[i], lhsT=lhsT,
                                    rhs=kv[:, g, 512 * lt:512 * (lt + 1)],
                                    start=(g == 0), stop=(g == G - 1))
                        for i in range(4):
                            lt = 4 * lh + i
                            ot = out_pool.tile([128, 512], BF16,
                                               name=f"ot{mo}_{lt}", tag="ot")
                            nc.scalar.add(ot, acc[i], bias_sb[:, mo:mo + 1])
                            nc.sync.dma_start(
                                out=out_d.ap()[128 * mo:128 * (mo + 1),
                                               512 * lt:512 * (lt + 1)],
                                in_=ot)
    nc.compile()
    return nc


def _get_nc():
    if "nc" not in _cache:
        _cache["nc"] = _build()
    return _cache["nc"]


def kernel(x, spatial_feat, multi_scale_feat, scale, conv_w, conv_b,
           _trace=False):
    from concourse.bass_utils import run_bass_kernel_spmd

    nc = _get_nc()
    x = np.ascontiguousarray(np.asarray(x, dtype=np.float32)).reshape(B, C, L)
    sp = np.ascontiguousarray(
        np.asarray(spatial_feat, dtype=np.float32)).reshape(B, C, L)
    ms = np.ascontiguousarray(
        np.asarray(multi_scale_feat, dtype=np.float32)).reshape(B, C, L)
    w = np.ascontiguousarray(np.asarray(conv_w, dtype=np.float32))
    bv = np.ascontiguousarray(np.asarray(conv_b, dtype=np.float32)).reshape(C)
    sc = np.asarray(scale, dtype=np.float32).reshape(1)

    in_maps = [{"q": x[b], "sp": sp[b], "ms": ms[b],
                "conv_w": w, "conv_b": bv, "scale": sc}
               for b in range(NCORES)]
    res = run_bass_kernel_spmd(nc, in_maps, core_ids=list(range(NCORES)),
                               trace=_trace)
    if _trace:
        _cache["last_result"] = res
    out = np.stack([np.asarray(res.results[b]["out"], dtype=np.float32)
                    for b in range(NCORES)])
    return out.reshape(B, C, H, W)
